# revision 31
# baseline (speedup 1.0000x reference)
"""Trainium2 Bass kernel for nn_CausalAttention (B=4, T=2048, d_model=1024, d_ff=2048).

Sharding: 8 cores = 4 batches x 2 query-halves. Each core owns 8 query blocks
of 128 rows (OWN_H), paired so causal work is balanced and the per-core
program is IDENTICAL (SPMD): the k-th owned block always computes E[k] key
chunks; exact causal masking arrives as per-core input data.

Math per core (fp32r = 4-byte fp32 storage, ~tf32 matmul precision, 1 PE
cycle/row at >=256-wide moving vs fp32's 4 - measured max|err| 0.028 on
score-scale tiles; bf16 is 0.53 and corrupts rows):
  M  = Wq @ Wk.T        (fp32r; c1-row slice per core, AllGather -> full M)
  N2 = Wv @ Wf          (bf16; g 256-col slice per core, AllGather -> full)
  uT = (xq M).T         (fp32r; owned query rows only)
  S  = uT.T @ x.T == q @ k.T  (fp32r scores; contracts d_model=1024 not d_ff)
  P' = softmax(S + mask) / rowsum  (exp on ScalarE, rescaled bf16 probs)
  y  = P' @ x           (bf16; attention output in INPUT space, owned rows)
  out= y @ N2 + bf      (bf16; = P'(x Wv) Wf by associativity)

The y-route is the key structural choice: out = (P' x) N2 instead of
P' (x N2).  y is computed only for the core's own 1024 query rows (1.2
GMAC) instead of materializing vf = x@N2 for all T rows (2.1 GMAC), and -
decisively - it needs NO exchange between pair cores (pair collectives
measured at only ~30-60 GB/s would otherwise gate the out phase). The only
collectives are three prelude AllGathers (M in two c2-halves, N2 full),
all hidden behind compute/loads.

fp32r stationary loads cannot overlap the running matmul (the 4-byte
weight load is folded into the MATMUL), so fp32r loops reuse each
stationary across as many moving matmuls as possible (uT: 2x, S: up to
4x); all bf16 phases reuse stationaries 2-4x as well.

DMA FIFO discipline (two HWDGE rings, FIFO per ring, order = emission
order): nc.sync carries input loads and collective-gated reads with
monotone gate times (mallA -> mallB -> n2all); nc.scalar carries the small
collective-input writes (msl, n2s) and output writes, so AllGather
triggers are never head-of-line blocked behind megabyte loads.
"""

import sys
from contextlib import ExitStack

for _p in ("/opt/trn_rl_repo", "/root/.axon_site/_ro/trn_rl_repo"):
    if _p not in sys.path:
        sys.path.append(_p)

import ml_dtypes
import numpy as np

import concourse.bass as bass
import concourse.mybir as mybir
import concourse.tile as tile
from concourse import bacc
from concourse.bass_utils import run_bass_kernel_spmd
from concourse.masks import make_identity

F32 = mybir.dt.float32
F32R = mybir.dt.float32r
BF16 = mybir.dt.bfloat16

B, T, C, F = 4, 2048, 1024, 2048
NB = T // 128  # 16 query/key blocks per batch
CC = C // 128  # 8 chunks of d_model
FC = F // 128  # 16 chunks of d_ff
NCORES = 8

# k-th owned block of each half; chosen so L(OWN_H[h][k]) <= E[k] for both h
# and sum(E)=72 (ideal causal: 68). E[k] = key chunks computed for block k.
OWN_H = {
    0: [15, 12, 11, 8, 7, 4, 3, 0],
    1: [14, 13, 10, 9, 6, 5, 2, 1],
}
E = [16, 14, 12, 10, 8, 6, 4, 2]
EOFF = [0, 16, 30, 42, 52, 60, 66, 70]  # prefix sums of E (ptsb tile offsets)
NPT = EOFF[7] + E[7]  # 72 transposed prob tiles
NEG = -1.0e30

ALL8 = [list(range(8))]

_CACHE = {}


def _build_program():
    """Trace + finalize the (single, SPMD) Bass program."""
    nc = bacc.Bacc(None)

    # all operands arrive pre-transposed / pre-cast / pre-sliced from the host
    # big operands arrive PRE-SHUFFLED partition-major [128, chunk, cols] so
    # every load is one fully-contiguous-per-partition DMA (~360 GB/s vs ~200
    # for the 4KB-strided chunk gather)
    xT_ext = nc.declare_dram_parameter("xTin", [128, CC, T], F32R, isOutput=False)
    xqT_ext = nc.declare_dram_parameter("xqTin", [128, CC, 1024], F32R, isOutput=False)
    xrow_ext = nc.declare_dram_parameter("xrowb", [128, NB, C], BF16, isOutput=False)
    m2_ext = nc.declare_dram_parameter("mask2", [128, 8, 256], BF16, isOutput=False)
    wkT_ext = nc.declare_dram_parameter("WkT", [128, FC, C], F32R, isOutput=False)
    wqs_ext = nc.declare_dram_parameter("WqTs", [128, FC, 128], F32R, isOutput=False)
    wvT_ext = nc.declare_dram_parameter("WvTb", [128, FC, C], BF16, isOutput=False)
    wfs_ext = nc.declare_dram_parameter("Wfs", [128, FC, 256], BF16, isOutput=False)
    bf_ext = nc.declare_dram_parameter("bf", [F], F32, isOutput=False)
    out_ext = nc.declare_dram_parameter("out", [8, 128, F], BF16, isOutput=True)

    with tile.TileContext(nc) as tc, ExitStack() as root:
        persist = root.enter_context(tc.tile_pool(name="persist", bufs=1))
        dram = root.enter_context(tc.tile_pool(name="dram", bufs=1, space="DRAM"))

        identbf = persist.tile([128, 128], BF16, tag="identbf")
        make_identity(nc, identbf[:, :])
        # long-lived operands (loads emitted late, where first needed)
        xT = persist.tile([128, CC, T], F32R, tag="xT")  # 64KB/part
        uT = persist.tile([128, CC, 1024], F32R, tag="uT")  # 32KB/part

        # collective buffers (DRAM)
        mslA_d = dram.tile([128, 512], F32R, tag="mslA_d")
        mslB_d = dram.tile([128, 512], F32R, tag="mslB_d")
        mallA_d = dram.tile([CC, 128, 512], F32R, tag="mallA_d", addr_space="Shared")
        mallB_d = dram.tile([CC, 128, 512], F32R, tag="mallB_d", addr_space="Shared")
        n2s_d = dram.tile([C, 256], BF16, tag="n2s_d")
        n2all_d = dram.tile(
            [NCORES, CC, 128, 256], BF16, tag="n2all_d", addr_space="Shared"
        )

        # ======== phase 1: N2-slice (compute only; its AllGather is
        # triggered after the M AllGathers to keep uT's gates first in the
        # global collective order) = Wv @ Wf[:, my g 256 cols], half-AG =====
        with ExitStack() as ph2:
            wvp = ph2.enter_context(tc.tile_pool(name="wvp", bufs=1))
            ps2 = ph2.enter_context(tc.tile_pool(name="ps2", bufs=1, space="PSUM"))
            wvT = wvp.tile([128, FC, C], BF16, tag="wvT")  # 32KB/part
            wfs = wvp.tile([128, FC, 256], BF16, tag="wfs")  # 8KB/part
            nc.sync.dma_start(out=wvT[:, :, :], in_=wvT_ext[:, :, :])
            nc.sync.dma_start(out=wfs[:, :, :], in_=wfs_ext[:, :, :])
            n2s_sb = wvp.tile([128, CC, 256], BF16, tag="n2s_sb")  # 4KB/part
            for ah in range(2):
                nps = ps2.tile([128, 4, 512], F32, tag="nps", name=f"nps{ah}")
                for f in range(FC):
                    for a4 in range(4):
                        nc.tensor.matmul(
                            nps[:, a4, :256],
                            wvT[:, f, (ah * 4 + a4) * 128 : (ah * 4 + a4 + 1) * 128],
                            wfs[:, f, :],
                            start=(f == 0),
                            stop=(f == FC - 1),
                        )
                for a4 in range(4):
                    nc.vector.tensor_copy(
                        out=n2s_sb[:, ah * 4 + a4, :], in_=nps[:, a4, :256]
                    )
            for a in range(CC):  # scalar FIFO: not blocked behind loads
                nc.scalar.dma_start(
                    out=n2s_d[a * 128 : (a + 1) * 128, :], in_=n2s_sb[:, a, :]
                )

        # ======== phase 2: M-slice = Wq.T[my c1 128 rows] @ WkT ==============
        # out [c1-128, c2-1024] fp32r at 512-wide moving; AllGather in two
        # c2-halves so uT can start on c2 0-3 one mesh earlier.
        with ExitStack() as ph1:
            wqp = ph1.enter_context(tc.tile_pool(name="wqp", bufs=1))
            ps1 = ph1.enter_context(tc.tile_pool(name="ps1", bufs=1, space="PSUM"))
            wkT = wqp.tile([128, FC, C], F32R, tag="wkT")  # 64KB/part
            wqs = wqp.tile([128, FC, 128], F32R, tag="wqs")  # 8KB/part
            nc.sync.dma_start(out=wkT[:, :, :], in_=wkT_ext[:, :, :])
            nc.sync.dma_start(out=wqs[:, :, :], in_=wqs_ext[:, :, :])
            msl_sb = wqp.tile([128, C], F32R, tag="msl_sb")  # 4KB/part
            mps = ps1.tile([128, 2, 512], F32, tag="mps")  # 2 banks
            for f in range(FC):
                for ch in range(2):
                    nc.tensor.matmul(
                        mps[:, ch, :],
                        wqs[:, f, :],
                        wkT[:, f, ch * 512 : (ch + 1) * 512],
                        start=(f == 0),
                        stop=(f == FC - 1),
                    )
            nc.vector.tensor_copy(out=msl_sb[:, :], in_=mps[:, :, :])
            nc.scalar.dma_start(out=mslA_d[:, :], in_=msl_sb[:, :512])
            nc.scalar.dma_start(out=mslB_d[:, :], in_=msl_sb[:, 512:])
            nc.gpsimd.collective_compute(
                "AllGather",
                mybir.AluOpType.bypass,
                replica_groups=ALL8,
                ins=[mslA_d[:, :]],
                outs=[mallA_d[:, :, :]],
            )
            nc.gpsimd.collective_compute(
                "AllGather",
                mybir.AluOpType.bypass,
                replica_groups=ALL8,
                ins=[mslB_d[:, :]],
                outs=[mallB_d[:, :, :]],
            )

        # N2 AllGather trigger (third in the global collective chain)
        nc.gpsimd.collective_compute(
            "AllGather",
            mybir.AluOpType.bypass,
            replica_groups=ALL8,
            ins=[n2s_d[:, :]],
            outs=[n2all_d[:, :, :, :]],
        )
        # pool spanning 5a .. out (transposed probs, transposed y, masks)
        atp = root.enter_context(tc.tile_pool(name="atp", bufs=1))
        ptsb = atp.tile([128, NPT, 128], BF16, tag="ptsb")  # 18KB/part
        yT = atp.tile([128, CC, 8, 128], BF16, tag="yT")  # 16KB/part
        rinv8 = atp.tile([128, 8], F32, tag="rinv8")  # per-block 1/rowsum

        # xqT load next on the sync FIFO (ungated; needed first by uT)
        xq_p = root.enter_context(ExitStack())
        xqp = xq_p.enter_context(tc.tile_pool(name="xqp", bufs=1))
        xqT = xqp.tile([128, CC, 1024], F32R, tag="xqT")  # 32KB/part
        nc.sync.dma_start(out=xqT[:, :, :], in_=xqT_ext[:, :, :])
        m2 = atp.tile([128, 8, 256], BF16, tag="m2")  # 4KB/part
        nc.sync.dma_start(out=m2[:, :, :], in_=m2_ext[:, :, :])

        # ======== phase 3: uT = (xq M).T  [c2-chunk, owned-t] fp32r ==========
        # M streamed per c2-chunk; each fp32r stationary feeds both tt movings
        with ExitStack() as ph3:
            mmp = ph3.enter_context(tc.tile_pool(name="mmp", bufs=2))
            ps3 = ph3.enter_context(tc.tile_pool(name="ps3", bufs=2, space="PSUM"))
            for c2 in range(CC):
                mMc = mmp.tile([128, CC, 128], F32R, tag="mMc", name=f"mMc{c2}")
                half, lo = (mallA_d, 0) if c2 < 4 else (mallB_d, 4)
                nc.sync.dma_start(
                    out=mMc[:, :, :],
                    in_=half[:, :, (c2 - lo) * 128 : (c2 - lo + 1) * 128].transpose(
                        [1, 0, 2]
                    ),
                )
                if c2 == 3:
                    # xT single load: after the mallA-gated reads, before the
                    # mallB gate; needed by 5a
                    nc.sync.dma_start(out=xT[:, :, :], in_=xT_ext[:, :, :])
                ups2 = ps3.tile([128, 2, 512], F32, tag="ups2", name=f"ups2_{c2}")
                for c1 in range(CC):
                    for tt in range(2):
                        nc.tensor.matmul(
                            ups2[:, tt, :],
                            mMc[:, c1, :],
                            xqT[:, c1, tt * 512 : (tt + 1) * 512],
                            start=(c1 == 0),
                            stop=(c1 == CC - 1),
                        )
                nc.vector.tensor_copy(out=uT[:, c2, :], in_=ups2[:, :, :])

        xq_p.close()  # xqT freed before 5a opens

        # ======== phase 4 (5a): scores + softmax + rescaled transposed probs =
        with ExitStack() as ph5:
            st5 = ph5.enter_context(tc.tile_pool(name="st5", bufs=2))
            small = ph5.enter_context(tc.tile_pool(name="small", bufs=4))
            ps5 = ph5.enter_context(tc.tile_pool(name="ps5", bufs=2, space="PSUM"))
            ps_t = ph5.enter_context(tc.tile_pool(name="ps_t", bufs=2, space="PSUM"))
            def softmax_stage(k):
                """scores -> +mask -> per-chunk max -> exp -> P/rowsum.

                Scores are computed in two 1024 PSUM chunks (double-buffered)
                with the row max reduced per chunk on DVE while the next
                chunk's matmuls run; exp uses the combined row max (a fixed
                shift is unsafe: early causal rows can have rowmax anywhere,
                and a whole-row exp underflow makes rowsum 0 -> NaN).
                """
                ek = E[k]
                scols = ek * 128
                psb = st5.tile([128, T], BF16, tag="psb", name=f"psb{k}", bufs=2)
                nm2 = small.tile([128, 2], F32, tag="nm2", name=f"nm2{k}", bufs=6)
                rs2 = small.tile([128, 2], F32, tag="rs2", name=f"rs2{k}", bufs=6)
                nchunk = (scols + 1023) // 1024
                chunks = []
                for ci in range(nchunk):
                    h0 = ci * 1024
                    hw = min(1024, scols - h0)
                    sps = ps5.tile(
                        [128, 1024], F32, tag="sps", name=f"sps{k}_{ci}"
                    )  # 2 banks
                    for c2 in range(CC):  # stationary reused across chunk tiles
                        for st in range(0, hw, 512):
                            w = min(512, hw - st)
                            nc.tensor.matmul(
                                sps[:, st : st + w],
                                uT[:, c2, k * 128 : (k + 1) * 128],
                                xT[:, c2, h0 + st : h0 + st + w],
                                start=(c2 == 0),
                                stop=(c2 == CC - 1),
                            )
                    if ci == nchunk - 1:  # mask window = last 256 score cols
                        nc.vector.tensor_add(
                            sps[:, hw - 256 : hw], sps[:, hw - 256 : hw], m2[:, k, :]
                        )
                    nc.vector.tensor_reduce(
                        out=nm2[:, ci : ci + 1],
                        in_=sps[:, :hw],
                        axis=mybir.AxisListType.X,
                        op=mybir.AluOpType.max,
                        negate=True,
                    )
                    chunks.append((sps, h0, hw))
                negmax = small.tile(
                    [128, 1], F32, tag="negmax", name=f"negmax{k}", bufs=6
                )
                if nchunk == 2:
                    nc.vector.tensor_tensor(
                        out=negmax,
                        in0=nm2[:, 0:1],
                        in1=nm2[:, 1:2],
                        op=mybir.AluOpType.min,
                    )
                else:
                    nc.vector.tensor_copy(out=negmax, in_=nm2[:, 0:1])
                for ci, (sps, h0, hw) in enumerate(chunks):
                    nc.scalar.activation(
                        out=psb[:, h0 : h0 + hw],
                        in_=sps[:, :hw],
                        func=mybir.ActivationFunctionType.Exp,
                        bias=negmax,
                        scale=1.0,
                        accum_out=rs2[:, ci : ci + 1],
                    )
                if nchunk == 2:
                    rsum = small.tile(
                        [128, 1], F32, tag="rsum", name=f"rsum{k}", bufs=6
                    )
                    nc.vector.tensor_add(rsum, rs2[:, 0:1], rs2[:, 1:2])
                    nc.vector.reciprocal(out=rinv8[:, k : k + 1], in_=rsum)
                else:
                    nc.vector.reciprocal(out=rinv8[:, k : k + 1], in_=rs2[:, 0:1])
                return psb

            def transpose_stage(k, psbS):
                for sc in range(E[k]):
                    pt = ps_t.tile([128, 128], BF16, tag="pt", name=f"pt{k}_{sc}")
                    nc.tensor.transpose(
                        pt[:, :], psbS[:, sc * 128 : (sc + 1) * 128], identbf[:, :]
                    )
                    nc.vector.tensor_copy(out=ptsb[:, EOFF[k] + sc, :], in_=pt[:, :])

            korder = list(range(7, -1, -1))  # small E first
            staged = {korder[0]: softmax_stage(korder[0])}
            for i, k in enumerate(korder):
                if i + 1 < 8:
                    staged[korder[i + 1]] = softmax_stage(korder[i + 1])
                transpose_stage(k, staged.pop(k))

        # ======== phase 5 (y): y = P' @ x  [own queries, d_model] bf16 ======
        # attention output in input space; transposed per c-chunk for phase 6
        with ExitStack() as ph5y:
            xrp = ph5y.enter_context(tc.tile_pool(name="xrp", bufs=1))
            yst = ph5y.enter_context(tc.tile_pool(name="yst", bufs=2))
            psy = ph5y.enter_context(tc.tile_pool(name="psy", bufs=2, space="PSUM"))
            ps_y = ph5y.enter_context(tc.tile_pool(name="ps_y", bufs=2, space="PSUM"))
            xrow = xrp.tile([128, NB, C], BF16, tag="xrow")  # 32KB/part
            nc.sync.dma_start(out=xrow[:, :, :], in_=xrow_ext[:, :, :])
            for k in range(7, -1, -1):  # matches 5a completion order
                ek = E[k]
                yps = psy.tile([128, C], F32, tag="yps", name=f"yps{k}")  # 2 banks
                for sc in range(ek):  # stationary P' tile reused 2x
                    for c2 in range(2):
                        nc.tensor.matmul(
                            yps[:, c2 * 512 : (c2 + 1) * 512],
                            ptsb[:, EOFF[k] + sc, :],
                            xrow[:, sc, c2 * 512 : (c2 + 1) * 512],
                            start=(sc == 0),
                            stop=(sc == ek - 1),
                        )
                ybf = yst.tile([128, C], BF16, tag="ybf", name=f"ybf{k}")
                nc.vector.tensor_copy(out=ybf[:, :], in_=yps[:, :])
                for cc in range(CC):
                    yt = ps_y.tile([128, 128], BF16, tag="yt", name=f"yt{k}_{cc}")
                    nc.tensor.transpose(
                        yt[:, :], ybf[:, cc * 128 : (cc + 1) * 128], identbf[:, :]
                    )
                    nc.vector.tensor_copy(out=yT[:, cc, k, :], in_=yt[:, :])

        # ======== phase 6: out = y @ N2 + bf  [own queries, full d_ff] =======
        with ExitStack() as ph6:
            n2p = ph6.enter_context(tc.tile_pool(name="n2p", bufs=1))
            orp = ph6.enter_context(tc.tile_pool(name="orp", bufs=2))
            ps6 = ph6.enter_context(tc.tile_pool(name="ps6", bufs=2, space="PSUM"))
            n2 = n2p.tile([128, CC, F], BF16, tag="n2")  # 32KB/part
            # n2all rows [g-core][cc][p][256] -> n2[p, cc, gi*256:...]
            for gi in range(NCORES):
                nc.sync.dma_start(
                    out=n2[:, :, gi * 256 : (gi + 1) * 256],
                    in_=n2all_d[gi].transpose([1, 0, 2]),
                )
            bfb = n2p.tile([128, F], F32, tag="bfb")  # 8KB/part
            bf_ap = bf_ext[:]
            nc.sync.dma_start(
                out=bfb,
                in_=bass.AP(
                    tensor=bf_ap.tensor,
                    offset=bf_ap.offset,
                    ap=[[0, 128]] + list(bf_ap.ap),
                ),
            )
            for k in range(8):
                ops = ps6.tile([128, F], F32, tag="ops", name=f"ops{k}")  # 4 banks
                for cc in range(CC):  # stationary yT tile reused 4x
                    for g4 in range(4):
                        nc.tensor.matmul(
                            ops[:, g4 * 512 : (g4 + 1) * 512],
                            yT[:, cc, k, :],
                            n2[:, cc, g4 * 512 : (g4 + 1) * 512],
                            start=(cc == 0),
                            stop=(cc == CC - 1),
                        )
                orow = orp.tile([128, F], BF16, tag="orow", name=f"orow{k}")
                nc.vector.scalar_tensor_tensor(
                    out=orow,
                    in0=ops,
                    scalar=rinv8[:, k : k + 1],
                    in1=bfb,
                    op0=mybir.AluOpType.mult,
                    op1=mybir.AluOpType.add,
                )
                nc.scalar.dma_start(out=out_ext[k], in_=orow)

    nc.finalize()
    return nc


def _get_program():
    if "nc" not in _CACHE:
        _CACHE["nc"] = _build_program()
    return _CACHE["nc"]


def _pm(a, chunks):
    """[chunks*128, cols] -> partition-major [128, chunks, cols]"""
    return np.ascontiguousarray(
        a.reshape(chunks, 128, a.shape[-1]).transpose(1, 0, 2)
    )


def _make_in_maps(x, Wq, Wk, Wv, Wf, bf):
    x = np.ascontiguousarray(x, dtype=np.float32)
    WqT = np.ascontiguousarray(np.asarray(Wq, dtype=np.float32).T)
    WkT = np.ascontiguousarray(np.asarray(Wk, dtype=np.float32).T)
    WvTb = np.ascontiguousarray(np.asarray(Wv, dtype=np.float32).T).astype(
        ml_dtypes.bfloat16
    )
    Wfb = np.asarray(Wf, dtype=np.float32).astype(ml_dtypes.bfloat16)
    bf = np.ascontiguousarray(bf, dtype=np.float32)
    in_maps = []
    for core in range(NCORES):
        b, h = core // 2, core % 2
        own = OWN_H[h]
        xb = x[b]
        xbT = np.ascontiguousarray(xb.T)
        xq = np.concatenate([xb[blk * 128 : (blk + 1) * 128] for blk in own], axis=0)
        xrowb = xb.astype(ml_dtypes.bfloat16)
        mask2 = np.zeros((8, 128, 256), dtype=np.float32)
        for k, blk in enumerate(own):
            s0 = (E[k] - 2) * 128  # global key index of mask window start
            s = s0 + np.arange(256)[None, :]
            t = blk * 128 + np.arange(128)[:, None]
            mask2[k] = np.where(s <= t, 0.0, NEG).astype(np.float32)
        in_maps.append(
            {
                "xTin": _pm(xbT, CC),
                "xqTin": _pm(np.ascontiguousarray(xq.T), CC),
                "xrowb": _pm(xrowb, NB),
                "mask2": np.ascontiguousarray(
                    mask2.astype(ml_dtypes.bfloat16).transpose(1, 0, 2)
                ),
                "WkT": _pm(WkT, FC),
                "WqTs": _pm(WqT[:, core * 128 : (core + 1) * 128], FC),
                "WvTb": _pm(WvTb, FC),
                "Wfs": _pm(Wfb[:, core * 256 : (core + 1) * 256], FC),
                "bf": bf,
            }
        )
    return in_maps


def run_on_hw(inputs, trace=False, trace_cores=None):
    nc = _get_program()
    in_maps = _make_in_maps(**inputs)
    res = run_bass_kernel_spmd(
        nc, in_maps, list(range(NCORES)), trace=trace, trace_cores=trace_cores
    )
    out = np.empty((B, T, F), dtype=np.float32)
    for core in range(NCORES):
        b, h = core // 2, core % 2
        o = res.results[core]["out"]  # [8, 128, F] bf16
        for k, blk in enumerate(OWN_H[h]):
            out[b, blk * 128 : (blk + 1) * 128, :] = o[k].astype(np.float32)
    return out, res


def kernel(x, Wq, Wk, Wv, Wf, bf):
    out, _ = run_on_hw(dict(x=x, Wq=Wq, Wk=Wk, Wv=Wv, Wf=Wf, bf=bf))
    return out


# revision 34
# speedup vs baseline: 1.0604x; 1.0604x over previous
"""Trainium2 Bass kernel for nn_CausalAttention (B=4, T=2048, d_model=1024, d_ff=2048).

Sharding: 8 cores = 4 batches x 2 query-halves. Each core owns 8 query blocks
of 128 rows (OWN_H), paired so causal work is balanced and the per-core
program is IDENTICAL (SPMD): the k-th owned block always computes E[k] key
chunks; exact causal masking arrives as per-core input data.

Math per core (fp32r = 4-byte fp32 storage, ~tf32 matmul precision, 1 PE
cycle/row at >=256-wide moving vs fp32's 4 - measured max|err| 0.028 on
score-scale tiles; bf16 is 0.53 and corrupts rows):
  M  = Wq @ Wk.T        (fp32r; c1-row slice per core, AllGather -> full M)
  N2 = Wv @ Wf          (bf16; g 256-col slice per core, AllGather -> full)
  uT = (xq M).T         (fp32r; owned query rows only)
  S  = uT.T @ x.T == q @ k.T  (fp32r scores; contracts d_model=1024 not d_ff)
  P' = softmax(S + mask) / rowsum  (exp on ScalarE, rescaled bf16 probs)
  y  = P' @ x           (bf16; attention output in INPUT space, owned rows)
  out= y @ N2 + bf      (bf16; = P'(x Wv) Wf by associativity)

The y-route is the key structural choice: out = (P' x) N2 instead of
P' (x N2).  y is computed only for the core's own 1024 query rows (1.2
GMAC) instead of materializing vf = x@N2 for all T rows (2.1 GMAC), and -
decisively - it needs NO exchange between pair cores (pair collectives
measured at only ~30-60 GB/s would otherwise gate the out phase). The only
collectives are three prelude AllGathers (M in two c2-halves, N2 full),
all hidden behind compute/loads.

fp32r stationary loads cannot overlap the running matmul (the 4-byte
weight load is folded into the MATMUL), so fp32r loops reuse each
stationary across as many moving matmuls as possible (uT: 2x, S: up to
4x); all bf16 phases reuse stationaries 2-4x as well.

DMA FIFO discipline (two HWDGE rings, FIFO per ring, order = emission
order): nc.sync carries input loads and collective-gated reads with
monotone gate times (mallA -> mallB -> n2all); nc.scalar carries the small
collective-input writes (msl, n2s) and output writes, so AllGather
triggers are never head-of-line blocked behind megabyte loads.
"""

import sys
from contextlib import ExitStack

for _p in ("/opt/trn_rl_repo", "/root/.axon_site/_ro/trn_rl_repo"):
    if _p not in sys.path:
        sys.path.append(_p)

import ml_dtypes
import numpy as np

import concourse.bass as bass
import concourse.mybir as mybir
import concourse.tile as tile
from concourse import bacc
from concourse.bass_utils import run_bass_kernel_spmd
from concourse.masks import make_identity

F32 = mybir.dt.float32
F32R = mybir.dt.float32r
BF16 = mybir.dt.bfloat16

B, T, C, F = 4, 2048, 1024, 2048
NB = T // 128  # 16 query/key blocks per batch
CC = C // 128  # 8 chunks of d_model
FC = F // 128  # 16 chunks of d_ff
NCORES = 8

# k-th owned block of each half; chosen so L(OWN_H[h][k]) <= E[k] for both h
# and sum(E)=72 (ideal causal: 68). E[k] = key chunks computed for block k.
OWN_H = {
    0: [15, 12, 11, 8, 7, 4, 3, 0],
    1: [14, 13, 10, 9, 6, 5, 2, 1],
}
E = [16, 14, 12, 10, 8, 6, 4, 2]
EOFF = [0, 16, 30, 42, 52, 60, 66, 70]  # prefix sums of E (ptsb tile offsets)
NPT = EOFF[7] + E[7]  # 72 transposed prob tiles
NEG = -1.0e30

ALL8 = [list(range(8))]

_CACHE = {}


def _build_program():
    """Trace + finalize the (single, SPMD) Bass program."""
    nc = bacc.Bacc(None)

    # all operands arrive pre-transposed / pre-cast / pre-sliced from the host
    # big operands arrive PRE-SHUFFLED partition-major [128, chunk, cols] so
    # every load is one fully-contiguous-per-partition DMA (~360 GB/s vs ~200
    # for the 4KB-strided chunk gather)
    xT_ext = nc.declare_dram_parameter("xTin", [128, CC, T], F32R, isOutput=False)
    xqT_ext = nc.declare_dram_parameter("xqTin", [128, CC, 1024], F32R, isOutput=False)
    xrow_ext = nc.declare_dram_parameter("xrowb", [128, NB, C], BF16, isOutput=False)
    m2_ext = nc.declare_dram_parameter("mask2", [128, 8, 256], BF16, isOutput=False)
    wkh_ext = nc.declare_dram_parameter("WkTh", [128, FC, C], BF16, isOutput=False)
    wkl_ext = nc.declare_dram_parameter("WkTl", [128, FC, C], BF16, isOutput=False)
    wqh_ext = nc.declare_dram_parameter("WqTsh", [128, FC, 128], BF16, isOutput=False)
    wql_ext = nc.declare_dram_parameter("WqTsl", [128, FC, 128], BF16, isOutput=False)
    wvT_ext = nc.declare_dram_parameter("WvTb", [128, FC, C], BF16, isOutput=False)
    wfs_ext = nc.declare_dram_parameter("Wfs", [128, FC, 256], BF16, isOutput=False)
    bf_ext = nc.declare_dram_parameter("bf", [F], F32, isOutput=False)
    out_ext = nc.declare_dram_parameter("out", [8, 128, F], BF16, isOutput=True)

    with tile.TileContext(nc) as tc, ExitStack() as root:
        persist = root.enter_context(tc.tile_pool(name="persist", bufs=1))
        dram = root.enter_context(tc.tile_pool(name="dram", bufs=1, space="DRAM"))

        identbf = persist.tile([128, 128], BF16, tag="identbf")
        make_identity(nc, identbf[:, :])
        # long-lived operands (loads emitted late, where first needed)
        xT = persist.tile([128, CC, T], F32R, tag="xT")  # 64KB/part
        uT = persist.tile([128, CC, 1024], F32R, tag="uT")  # 32KB/part

        # collective buffers (DRAM)
        mslA_d = dram.tile([128, 512], F32R, tag="mslA_d")
        mslB_d = dram.tile([128, 512], F32R, tag="mslB_d")
        mallA_d = dram.tile([CC, 128, 512], F32R, tag="mallA_d", addr_space="Shared")
        mallB_d = dram.tile([CC, 128, 512], F32R, tag="mallB_d", addr_space="Shared")
        n2s_d = dram.tile([C, 256], BF16, tag="n2s_d")
        n2all_d = dram.tile(
            [NCORES, CC, 128, 256], BF16, tag="n2all_d", addr_space="Shared"
        )

        # ======== phase 1: M-slice = Wq.T[my c1 128 rows] @ WkT ==============
        # out [c1-128, c2-1024] fp32r at 512-wide moving; AllGather in two
        # c2-halves so uT can start on c2 0-3 one mesh earlier.
        with ExitStack() as ph1:
            wqp = ph1.enter_context(tc.tile_pool(name="wqp", bufs=1))
            ps1 = ph1.enter_context(tc.tile_pool(name="ps1", bufs=1, space="PSUM"))
            wkh = wqp.tile([128, FC, C], BF16, tag="wkh")  # 32KB/part
            wkl = wqp.tile([128, FC, C], BF16, tag="wkl")  # 32KB/part
            wqh = wqp.tile([128, FC, 128], BF16, tag="wqh")  # 4KB/part
            wql = wqp.tile([128, FC, 128], BF16, tag="wql")  # 4KB/part
            nc.sync.dma_start(out=wkh[:, :, :], in_=wkh_ext[:, :, :])
            nc.sync.dma_start(out=wqh[:, :, :], in_=wqh_ext[:, :, :])
            nc.sync.dma_start(out=wkl[:, :, :], in_=wkl_ext[:, :, :])
            nc.sync.dma_start(out=wql[:, :, :], in_=wql_ext[:, :, :])
            msl_sb = wqp.tile([128, C], F32R, tag="msl_sb")  # 4KB/part
            mps = ps1.tile([128, C], F32, tag="mps")  # 2 banks
            # split-bf16 fp32 emulation: qh kh + qh kl + ql kh (lo*lo dropped);
            # measured max|err| 9e-4 at score scale vs fp32r's 2.8e-2
            for f in range(FC):
                for qw, kw, st, sp in (
                    (wqh, wkh, f == 0, False),
                    (wqh, wkl, False, False),
                    (wql, wkh, False, f == FC - 1),
                ):
                    for ch in range(2):
                        nc.tensor.matmul(
                            mps[:, ch * 512 : (ch + 1) * 512],
                            qw[:, f, :],
                            kw[:, f, ch * 512 : (ch + 1) * 512],
                            start=st,
                            stop=sp,
                        )
            nc.vector.tensor_copy(out=msl_sb[:, :], in_=mps[:, :])
            nc.scalar.dma_start(out=mslA_d[:, :], in_=msl_sb[:, :512])
            nc.scalar.dma_start(out=mslB_d[:, :], in_=msl_sb[:, 512:])
            nc.gpsimd.collective_compute(
                "AllGather",
                mybir.AluOpType.bypass,
                replica_groups=ALL8,
                ins=[mslA_d[:, :]],
                outs=[mallA_d[:, :, :]],
            )
            nc.gpsimd.collective_compute(
                "AllGather",
                mybir.AluOpType.bypass,
                replica_groups=ALL8,
                ins=[mslB_d[:, :]],
                outs=[mallB_d[:, :, :]],
            )

        # ======== phase 2: N2-slice = Wv @ Wf[:, my g 256 cols], half-AG =====
        with ExitStack() as ph2:
            wvp = ph2.enter_context(tc.tile_pool(name="wvp", bufs=1))
            ps2 = ph2.enter_context(tc.tile_pool(name="ps2", bufs=1, space="PSUM"))
            wvT = wvp.tile([128, FC, C], BF16, tag="wvT")  # 32KB/part
            wfs = wvp.tile([128, FC, 256], BF16, tag="wfs")  # 8KB/part
            nc.sync.dma_start(out=wvT[:, :, :], in_=wvT_ext[:, :, :])
            nc.sync.dma_start(out=wfs[:, :, :], in_=wfs_ext[:, :, :])
            n2s_sb = wvp.tile([128, CC, 256], BF16, tag="n2s_sb")  # 4KB/part
            for ah in range(2):
                nps = ps2.tile([128, 4, 512], F32, tag="nps", name=f"nps{ah}")
                for f in range(FC):
                    for a4 in range(4):
                        nc.tensor.matmul(
                            nps[:, a4, :256],
                            wvT[:, f, (ah * 4 + a4) * 128 : (ah * 4 + a4 + 1) * 128],
                            wfs[:, f, :],
                            start=(f == 0),
                            stop=(f == FC - 1),
                        )
                for a4 in range(4):
                    nc.vector.tensor_copy(
                        out=n2s_sb[:, ah * 4 + a4, :], in_=nps[:, a4, :256]
                    )
            for a in range(CC):  # scalar FIFO: not blocked behind loads
                nc.scalar.dma_start(
                    out=n2s_d[a * 128 : (a + 1) * 128, :], in_=n2s_sb[:, a, :]
                )
            nc.gpsimd.collective_compute(
                "AllGather",
                mybir.AluOpType.bypass,
                replica_groups=ALL8,
                ins=[n2s_d[:, :]],
                outs=[n2all_d[:, :, :, :]],
            )

        # pool spanning 5a .. out (transposed probs, transposed y, masks)
        atp = root.enter_context(tc.tile_pool(name="atp", bufs=1))
        ptsb = atp.tile([128, NPT, 128], BF16, tag="ptsb")  # 18KB/part
        yT = atp.tile([128, CC, 8, 128], BF16, tag="yT")  # 16KB/part
        rinv8 = atp.tile([128, 8], F32, tag="rinv8")  # per-block 1/rowsum

        # xqT load next on the sync FIFO (ungated; needed first by uT)
        xq_p = root.enter_context(ExitStack())
        xqp = xq_p.enter_context(tc.tile_pool(name="xqp", bufs=1))
        xqT = xqp.tile([128, CC, 1024], F32R, tag="xqT")  # 32KB/part
        nc.sync.dma_start(out=xqT[:, :, :], in_=xqT_ext[:, :, :])
        m2 = atp.tile([128, 8, 256], BF16, tag="m2")  # 4KB/part
        nc.sync.dma_start(out=m2[:, :, :], in_=m2_ext[:, :, :])

        # ======== phase 3: uT = (xq M).T  [c2-chunk, owned-t] fp32r ==========
        # M streamed per c2-chunk; each fp32r stationary feeds both tt movings
        with ExitStack() as ph3:
            mmp = ph3.enter_context(tc.tile_pool(name="mmp", bufs=2))
            ps3 = ph3.enter_context(tc.tile_pool(name="ps3", bufs=2, space="PSUM"))
            for c2 in range(CC):
                mMc = mmp.tile([128, CC, 128], F32R, tag="mMc", name=f"mMc{c2}")
                half, lo = (mallA_d, 0) if c2 < 4 else (mallB_d, 4)
                nc.sync.dma_start(
                    out=mMc[:, :, :],
                    in_=half[:, :, (c2 - lo) * 128 : (c2 - lo + 1) * 128].transpose(
                        [1, 0, 2]
                    ),
                )
                if c2 == 3:
                    # xT single load: after the mallA-gated reads, before the
                    # mallB gate; needed by 5a
                    nc.sync.dma_start(out=xT[:, :, :], in_=xT_ext[:, :, :])
                ups2 = ps3.tile([128, 2, 512], F32, tag="ups2", name=f"ups2_{c2}")
                for c1 in range(CC):
                    for tt in range(2):
                        nc.tensor.matmul(
                            ups2[:, tt, :],
                            mMc[:, c1, :],
                            xqT[:, c1, tt * 512 : (tt + 1) * 512],
                            start=(c1 == 0),
                            stop=(c1 == CC - 1),
                        )
                nc.vector.tensor_copy(out=uT[:, c2, :], in_=ups2[:, :, :])

        xq_p.close()  # xqT freed before 5a opens

        # ======== phase 4 (5a): scores + softmax + rescaled transposed probs =
        with ExitStack() as ph5:
            st5 = ph5.enter_context(tc.tile_pool(name="st5", bufs=2))
            small = ph5.enter_context(tc.tile_pool(name="small", bufs=4))
            ps5 = ph5.enter_context(tc.tile_pool(name="ps5", bufs=2, space="PSUM"))
            ps_t = ph5.enter_context(tc.tile_pool(name="ps_t", bufs=2, space="PSUM"))
            def softmax_stage(k):
                """scores -> +mask -> per-chunk max -> exp -> P/rowsum.

                Scores are computed in two 1024 PSUM chunks (double-buffered)
                with the row max reduced per chunk on DVE while the next
                chunk's matmuls run; exp uses the combined row max (a fixed
                shift is unsafe: early causal rows can have rowmax anywhere,
                and a whole-row exp underflow makes rowsum 0 -> NaN).
                """
                ek = E[k]
                scols = ek * 128
                psb = st5.tile([128, T], BF16, tag="psb", name=f"psb{k}", bufs=2)
                nm2 = small.tile([128, 2], F32, tag="nm2", name=f"nm2{k}", bufs=6)
                rs2 = small.tile([128, 2], F32, tag="rs2", name=f"rs2{k}", bufs=6)
                nchunk = (scols + 1023) // 1024
                chunks = []
                for ci in range(nchunk):
                    h0 = ci * 1024
                    hw = min(1024, scols - h0)
                    sps = ps5.tile(
                        [128, 1024], F32, tag="sps", name=f"sps{k}_{ci}"
                    )  # 2 banks
                    for c2 in range(CC):  # stationary reused across chunk tiles
                        for st in range(0, hw, 512):
                            w = min(512, hw - st)
                            nc.tensor.matmul(
                                sps[:, st : st + w],
                                uT[:, c2, k * 128 : (k + 1) * 128],
                                xT[:, c2, h0 + st : h0 + st + w],
                                start=(c2 == 0),
                                stop=(c2 == CC - 1),
                            )
                    if ci == nchunk - 1:  # mask window = last 256 score cols
                        nc.vector.tensor_add(
                            sps[:, hw - 256 : hw], sps[:, hw - 256 : hw], m2[:, k, :]
                        )
                    nc.vector.tensor_reduce(
                        out=nm2[:, ci : ci + 1],
                        in_=sps[:, :hw],
                        axis=mybir.AxisListType.X,
                        op=mybir.AluOpType.max,
                        negate=True,
                    )
                    chunks.append((sps, h0, hw))
                negmax = small.tile(
                    [128, 1], F32, tag="negmax", name=f"negmax{k}", bufs=6
                )
                if nchunk == 2:
                    nc.vector.tensor_tensor(
                        out=negmax,
                        in0=nm2[:, 0:1],
                        in1=nm2[:, 1:2],
                        op=mybir.AluOpType.min,
                    )
                else:
                    nc.vector.tensor_copy(out=negmax, in_=nm2[:, 0:1])
                for ci, (sps, h0, hw) in enumerate(chunks):
                    nc.scalar.activation(
                        out=psb[:, h0 : h0 + hw],
                        in_=sps[:, :hw],
                        func=mybir.ActivationFunctionType.Exp,
                        bias=negmax,
                        scale=1.0,
                        accum_out=rs2[:, ci : ci + 1],
                    )
                if nchunk == 2:
                    rsum = small.tile(
                        [128, 1], F32, tag="rsum", name=f"rsum{k}", bufs=6
                    )
                    nc.vector.tensor_add(rsum, rs2[:, 0:1], rs2[:, 1:2])
                    nc.vector.reciprocal(out=rinv8[:, k : k + 1], in_=rsum)
                else:
                    nc.vector.reciprocal(out=rinv8[:, k : k + 1], in_=rs2[:, 0:1])
                return psb

            def transpose_stage(k, psbS):
                for sc in range(E[k]):
                    pt = ps_t.tile([128, 128], BF16, tag="pt", name=f"pt{k}_{sc}")
                    nc.tensor.transpose(
                        pt[:, :], psbS[:, sc * 128 : (sc + 1) * 128], identbf[:, :]
                    )
                    nc.vector.tensor_copy(out=ptsb[:, EOFF[k] + sc, :], in_=pt[:, :])

            korder = list(range(7, -1, -1))  # small E first
            staged = {korder[0]: softmax_stage(korder[0])}
            for i, k in enumerate(korder):
                if i + 1 < 8:
                    staged[korder[i + 1]] = softmax_stage(korder[i + 1])
                transpose_stage(k, staged.pop(k))

        # ======== phase 5 (y): y = P' @ x  [own queries, d_model] bf16 ======
        # attention output in input space; transposed per c-chunk for phase 6
        with ExitStack() as ph5y:
            xrp = ph5y.enter_context(tc.tile_pool(name="xrp", bufs=1))
            yst = ph5y.enter_context(tc.tile_pool(name="yst", bufs=2))
            psy = ph5y.enter_context(tc.tile_pool(name="psy", bufs=2, space="PSUM"))
            ps_y = ph5y.enter_context(tc.tile_pool(name="ps_y", bufs=2, space="PSUM"))
            xrow = xrp.tile([128, NB, C], BF16, tag="xrow")  # 32KB/part
            nc.sync.dma_start(out=xrow[:, :, :], in_=xrow_ext[:, :, :])
            for k in range(7, -1, -1):  # matches 5a completion order
                ek = E[k]
                yps = psy.tile([128, C], F32, tag="yps", name=f"yps{k}")  # 2 banks
                for sc in range(ek):  # stationary P' tile reused 2x
                    for c2 in range(2):
                        nc.tensor.matmul(
                            yps[:, c2 * 512 : (c2 + 1) * 512],
                            ptsb[:, EOFF[k] + sc, :],
                            xrow[:, sc, c2 * 512 : (c2 + 1) * 512],
                            start=(sc == 0),
                            stop=(sc == ek - 1),
                        )
                ybf = yst.tile([128, C], BF16, tag="ybf", name=f"ybf{k}")
                nc.vector.tensor_copy(out=ybf[:, :], in_=yps[:, :])
                for cc in range(CC):
                    yt = ps_y.tile([128, 128], BF16, tag="yt", name=f"yt{k}_{cc}")
                    nc.tensor.transpose(
                        yt[:, :], ybf[:, cc * 128 : (cc + 1) * 128], identbf[:, :]
                    )
                    nc.vector.tensor_copy(out=yT[:, cc, k, :], in_=yt[:, :])

        # ======== phase 6: out = y @ N2 + bf  [own queries, full d_ff] =======
        with ExitStack() as ph6:
            n2p = ph6.enter_context(tc.tile_pool(name="n2p", bufs=1))
            orp = ph6.enter_context(tc.tile_pool(name="orp", bufs=2))
            ps6 = ph6.enter_context(tc.tile_pool(name="ps6", bufs=2, space="PSUM"))
            n2 = n2p.tile([128, CC, F], BF16, tag="n2")  # 32KB/part
            # n2all rows [g-core][cc][p][256] -> n2[p, cc, gi*256:...]
            for gi in range(NCORES):
                nc.sync.dma_start(
                    out=n2[:, :, gi * 256 : (gi + 1) * 256],
                    in_=n2all_d[gi].transpose([1, 0, 2]),
                )
            bfb = n2p.tile([128, F], F32, tag="bfb")  # 8KB/part
            bf_ap = bf_ext[:]
            nc.sync.dma_start(
                out=bfb,
                in_=bass.AP(
                    tensor=bf_ap.tensor,
                    offset=bf_ap.offset,
                    ap=[[0, 128]] + list(bf_ap.ap),
                ),
            )
            for k in range(8):
                ops = ps6.tile([128, F], F32, tag="ops", name=f"ops{k}")  # 4 banks
                for cc in range(CC):  # stationary yT tile reused 4x
                    for g4 in range(4):
                        nc.tensor.matmul(
                            ops[:, g4 * 512 : (g4 + 1) * 512],
                            yT[:, cc, k, :],
                            n2[:, cc, g4 * 512 : (g4 + 1) * 512],
                            start=(cc == 0),
                            stop=(cc == CC - 1),
                        )
                orow = orp.tile([128, F], BF16, tag="orow", name=f"orow{k}")
                nc.vector.scalar_tensor_tensor(
                    out=orow,
                    in0=ops,
                    scalar=rinv8[:, k : k + 1],
                    in1=bfb,
                    op0=mybir.AluOpType.mult,
                    op1=mybir.AluOpType.add,
                )
                nc.scalar.dma_start(out=out_ext[k], in_=orow)

    nc.finalize()
    return nc


def _get_program():
    if "nc" not in _CACHE:
        _CACHE["nc"] = _build_program()
    return _CACHE["nc"]


def _pm(a, chunks):
    """[chunks*128, cols] -> partition-major [128, chunks, cols]"""
    return np.ascontiguousarray(
        a.reshape(chunks, 128, a.shape[-1]).transpose(1, 0, 2)
    )


def _make_in_maps(x, Wq, Wk, Wv, Wf, bf):
    x = np.ascontiguousarray(x, dtype=np.float32)
    WqT = np.ascontiguousarray(np.asarray(Wq, dtype=np.float32).T)
    WkT = np.ascontiguousarray(np.asarray(Wk, dtype=np.float32).T)
    WkTh = WkT.astype(ml_dtypes.bfloat16)
    WkTl = (WkT - WkTh.astype(np.float32)).astype(ml_dtypes.bfloat16)
    WvTb = np.ascontiguousarray(np.asarray(Wv, dtype=np.float32).T).astype(
        ml_dtypes.bfloat16
    )
    Wfb = np.asarray(Wf, dtype=np.float32).astype(ml_dtypes.bfloat16)
    bf = np.ascontiguousarray(bf, dtype=np.float32)
    in_maps = []
    for core in range(NCORES):
        b, h = core // 2, core % 2
        own = OWN_H[h]
        xb = x[b]
        xbT = np.ascontiguousarray(xb.T)
        xq = np.concatenate([xb[blk * 128 : (blk + 1) * 128] for blk in own], axis=0)
        xrowb = xb.astype(ml_dtypes.bfloat16)
        Wqs = WqT[:, core * 128 : (core + 1) * 128]
        Wqsh = Wqs.astype(ml_dtypes.bfloat16)
        Wqsl = (Wqs - Wqsh.astype(np.float32)).astype(ml_dtypes.bfloat16)
        mask2 = np.zeros((8, 128, 256), dtype=np.float32)
        for k, blk in enumerate(own):
            s0 = (E[k] - 2) * 128  # global key index of mask window start
            s = s0 + np.arange(256)[None, :]
            t = blk * 128 + np.arange(128)[:, None]
            mask2[k] = np.where(s <= t, 0.0, NEG).astype(np.float32)
        in_maps.append(
            {
                "xTin": _pm(xbT, CC),
                "xqTin": _pm(np.ascontiguousarray(xq.T), CC),
                "xrowb": _pm(xrowb, NB),
                "mask2": np.ascontiguousarray(
                    mask2.astype(ml_dtypes.bfloat16).transpose(1, 0, 2)
                ),
                "WkTh": _pm(WkTh, FC),
                "WkTl": _pm(WkTl, FC),
                "WqTsh": _pm(Wqsh, FC),
                "WqTsl": _pm(Wqsl, FC),
                "WvTb": _pm(WvTb, FC),
                "Wfs": _pm(Wfb[:, core * 256 : (core + 1) * 256], FC),
                "bf": bf,
            }
        )
    return in_maps


def run_on_hw(inputs, trace=False, trace_cores=None):
    nc = _get_program()
    in_maps = _make_in_maps(**inputs)
    res = run_bass_kernel_spmd(
        nc, in_maps, list(range(NCORES)), trace=trace, trace_cores=trace_cores
    )
    out = np.empty((B, T, F), dtype=np.float32)
    for core in range(NCORES):
        b, h = core // 2, core % 2
        o = res.results[core]["out"]  # [8, 128, F] bf16
        for k, blk in enumerate(OWN_H[h]):
            out[b, blk * 128 : (blk + 1) * 128, :] = o[k].astype(np.float32)
    return out, res


def kernel(x, Wq, Wk, Wv, Wf, bf):
    out, _ = run_on_hw(dict(x=x, Wq=Wq, Wk=Wk, Wv=Wv, Wf=Wf, bf=bf))
    return out
